# revision 1
# baseline (speedup 1.0000x reference)
"""Trainium2 Bass kernel for nn_GCN_23029614641773.

The reference GCN operates on B independent 27-node graphs where every node of
graph i starts with the same feature vector x[i], and only node 0 of each graph
feeds the classifier head. Exploiting linearity of the edge aggregation, the
whole network collapses exactly (up to fp rounding order) to a per-sample MLP:

    y = x @ W0                                  # [B, 1024]
    s = lrelu(y + b0) + 2*lrelu(3y + b0) + lrelu(5y + b0)
      # node 1's in-neighbours {0,2,4,6} have in-degrees {1,3,3,5};
      # 2*lrelu(3y+b0) == lrelu(6y+2*b0) exactly (scaling by 2 is exact).
      # With b0 == 0 (spec fill): s == max(12y, 2.4y) exactly.
    t = s @ W1;  h = lrelu(t + b1)              # [B, 512]
    v = h @ W2;  g = lrelu(v + b2)              # [B, 256]
    out = g @ Wc + bc                           # [B, 1]

Sharding: pure data parallelism, batch split across 8 NeuronCores; each core
holds the full weight set.

Layout on device: activations kept transposed (features on partitions, batch
on the free dim) so every layer is matmul(out_T, lhsT=W_chunk, rhs=act_T) with
K accumulated in PSUM. x is transposed once on-chip via PE transposes.
"""

from contextlib import ExitStack

import numpy as np

import concourse.bacc as bacc
import concourse.mybir as mybir
import concourse.tile as tile
from concourse.bass_utils import run_bass_kernel_spmd

F32 = mybir.dt.float32
P = 128
N_CORES = 8
B_FULL = 2048
B = B_FULL // N_CORES  # 256 rows per core
D0, D1, D2, D3 = 1024, 1024, 512, 256
K0, M0 = D0 // P, D1 // P  # 8, 8
K1, M1 = D1 // P, D2 // P  # 8, 4
K2, M2 = D2 // P, D3 // P  # 4, 2
KC = D3 // P  # 2

NEG_SLOPE = 0.2
USE_F32R = True  # stream matmuls as float32r (4x faster on TRN2 PE)
F32R = mybir.dt.float32r


def _mm(ap):
    return ap.bitcast(F32R) if USE_F32R else ap


def _build(zero_bias: bool):
    nc = bacc.Bacc(
        "TRN2", target_bir_lowering=False, debug=False,
        enable_asserts=False, num_devices=1,
    )

    x_d = nc.dram_tensor("x", [B, D0], F32, kind="ExternalInput").ap()
    w0_d = nc.dram_tensor("W0", [D0, D1], F32, kind="ExternalInput").ap()
    b0_d = nc.dram_tensor("b0", [D1], F32, kind="ExternalInput").ap()
    w1_d = nc.dram_tensor("W1", [D1, D2], F32, kind="ExternalInput").ap()
    b1_d = nc.dram_tensor("b1", [D2], F32, kind="ExternalInput").ap()
    w2_d = nc.dram_tensor("W2", [D2, D3], F32, kind="ExternalInput").ap()
    b2_d = nc.dram_tensor("b2", [D3], F32, kind="ExternalInput").ap()
    wc_d = nc.dram_tensor("Wc", [D3, 1], F32, kind="ExternalInput").ap()
    bc_d = nc.dram_tensor("bc", [1], F32, kind="ExternalInput").ap()
    eye_d = nc.dram_tensor("eye", [P, P], F32, kind="ExternalInput").ap()
    out_d = nc.dram_tensor("out", [1, B], F32, kind="ExternalOutput").ap()

    with ExitStack() as ctx:
        tc = ctx.enter_context(tile.TileContext(nc))
        const = ctx.enter_context(tc.tile_pool(name="const", bufs=1))
        xrow_p = ctx.enter_context(tc.tile_pool(name="xrow", bufs=2))
        xt_p = ctx.enter_context(tc.tile_pool(name="xt", bufs=K0))
        w0_p = ctx.enter_context(tc.tile_pool(name="w0", bufs=K0 // 2))
        w1_p = ctx.enter_context(tc.tile_pool(name="w1", bufs=K1 // 2))
        w2_p = ctx.enter_context(tc.tile_pool(name="w2", bufs=K2 // 2))
        wc_p = ctx.enter_context(tc.tile_pool(name="wc", bufs=1))
        s_p = ctx.enter_context(tc.tile_pool(name="s", bufs=K1))
        h_p = ctx.enter_context(tc.tile_pool(name="h", bufs=K2))
        g_p = ctx.enter_context(tc.tile_pool(name="g", bufs=KC))
        tmp_p = ctx.enter_context(tc.tile_pool(name="tmp", bufs=4))
        out_p = ctx.enter_context(tc.tile_pool(name="outp", bufs=1))
        ps_p = ctx.enter_context(tc.tile_pool(name="ps", bufs=7, space="PSUM"))
        cls_ps = ctx.enter_context(tc.tile_pool(name="cls", bufs=1, space="PSUM"))

        # leaky-relu slope as a per-partition alpha vector for ACT Prelu
        alt = const.tile([P, 1], F32, tag="alt")
        nc.vector.memset(alt[:], NEG_SLOPE)

        # ---- DMA order = HBM arrival order: eye + x first (feed the
        # transposes), then W0 (gates layer 1), W1, W2, Wc. All big loads on
        # the sync HWDGE ring; scalar ring stays free for activations. ----
        eye = const.tile([P, P], F32, tag="eye")
        nc.scalar.dma_start(eye[:], eye_d)
        xr = []
        xpair = xrow_p.tile([P, 2 * D0], F32, tag="xr", name="xpair")
        nc.sync.dma_start(xpair[:], x_d.rearrange("(c p) m -> p c m", p=P))
        for r in range(B // P):
            xr.append(xpair[:, r * D0:(r + 1) * D0])

        # W as contraction-chunk row tiles: chunk c = W[c*128:(c+1)*128, :]
        # (contiguous rows -> cheap DMA descriptors); lhsT for (c, m) is
        # chunk_c[:, m*128:(m+1)*128]
        def row_chunks(pool, w_dram, K, N):
            # pairs of contraction chunks per DMA (halves the issue count;
            # per-partition runs stay contiguous at N*4 bytes)
            chunks = []
            G = 2
            for i in range(K // G):
                t = pool.tile([P, G * N], F32, tag="w",
                              name=f"wgrp_{w_dram.tensor.name}_{i}")
                src_ap = w_dram[G * i * P:(G * i + G) * P, :].rearrange(
                    "(c p) m -> p c m", p=P)
                nc.sync.dma_start(_mm(t[:]), _mm(src_ap))
                for j in range(G):
                    chunks.append(t[:, j * N:(j + 1) * N])
            return chunks

        w0 = row_chunks(w0_p, w0_d, K0, D1)
        w1 = row_chunks(w1_p, w1_d, K1, D2)
        w2 = row_chunks(w2_p, w2_d, K2, D3)
        wc = wc_p.tile([P, KC], F32)
        nc.sync.dma_start(_mm(wc[:]), _mm(wc_d.rearrange("(c p) j -> p c j", p=P)))

        if not zero_bias:
            b0t = const.tile([P, M0], F32, tag="b0t")
            nc.scalar.dma_start(b0t[:], b0_d.rearrange("(c p) -> p c", p=P))
            b1t = const.tile([P, M1], F32, tag="b1t")
            nc.scalar.dma_start(b1t[:], b1_d.rearrange("(c p) -> p c", p=P))
            b2t = const.tile([P, M2], F32, tag="b2t")
            nc.scalar.dma_start(b2t[:], b2_d.rearrange("(c p) -> p c", p=P))
            bct = const.tile([1, 1], F32, tag="bct")
            nc.scalar.dma_start(bct[:], bc_d.rearrange("(a b) -> a b", a=1))
            b0t2 = const.tile([P, M0], F32, tag="b0t2")
            nc.vector.tensor_scalar_mul(b0t2[:], b0t[:], 2.0)

        # ---- transpose x: [256, 1024] -> 8 tiles [128, 256] ----
        xt = []
        for k in range(K0):
            xtk = xt_p.tile([P, B], F32, tag="xt", name=f"xt_{k}")
            for r in range(B // P):
                pt = ps_p.tile([P, P], F32, tag="ps", name=f"pt_{k}_{r}")
                nc.tensor.transpose(pt[:], xr[r][:, k * P:(k + 1) * P], eye[:])
                nc.vector.tensor_copy(_mm(xtk[:, r * P:(r + 1) * P]), pt[:])
            xt.append(xtk)

        PRELU = mybir.ActivationFunctionType.Prelu

        def matmul_group(ps, chunks, m, rhs_tiles, K, rot=0):
            order = [(c + rot) % K for c in range(K)]
            for i, c in enumerate(order):
                nc.tensor.matmul(
                    ps[:], lhsT=_mm(chunks[c][:, m * P:(m + 1) * P]),
                    rhs=_mm(rhs_tiles[c][:]),
                    start=(i == 0), stop=(i == K - 1),
                )

        # ---- layer 1: y[m] = sum_c W0[c,m].T @ xT[c];
        #      s = 12*lrelu(y) = Prelu(12*y) exactly (zero bias) ----
        s_tiles = []
        for m in range(M0):
            ps = ps_p.tile([P, B], F32, tag="ps", name=f"ps1_{m}")
            matmul_group(ps, w0, m, xt, K0)
            s = s_p.tile([P, B], F32, tag="s", name=f"s_{m}")
            if zero_bias:
                nc.scalar.activation(_mm(s[:]), ps[:], PRELU,
                                     scale=12.0, alpha=alt[:])
            else:
                first = True
                for scale, bias in ((1.0, b0t[:, m:m + 1]), (6.0, b0t2[:, m:m + 1]),
                                    (5.0, b0t[:, m:m + 1])):
                    l = tmp_p.tile([P, B], F32, tag="l", name=f"l_{m}")
                    nc.scalar.activation(l[:], ps[:], PRELU,
                                         scale=scale, bias=bias, alpha=alt[:])
                    if first:
                        nc.vector.tensor_copy(_mm(s[:]), l[:])
                        first = False
                    else:
                        nc.vector.tensor_add(_mm(s[:]), _mm(s[:]), l[:])
            s_tiles.append(s)

        # ---- layer 2: t[m] = sum_c W1[c,m].T @ s[c]; h = lrelu(t + b1) ----
        h_tiles = []
        for m in range(M1):
            ps = ps_p.tile([P, B], F32, tag="ps", name=f"ps2_{m}")
            matmul_group(ps, w1, m, s_tiles, K1)
            h = h_p.tile([P, B], F32, tag="h", name=f"h_{m}")
            if zero_bias:
                nc.scalar.activation(_mm(h[:]), ps[:], PRELU, alpha=alt[:])
            else:
                nc.scalar.activation(_mm(h[:]), ps[:], PRELU,
                                     bias=b1t[:, m:m + 1], alpha=alt[:])
            h_tiles.append(h)

        # ---- layer 3: v[m] = sum_c W2[c,m].T @ h[c]; g = lrelu(v + b2) ----
        g_tiles = []
        for m in range(M2):
            ps = ps_p.tile([P, B], F32, tag="ps", name=f"ps3_{m}")
            matmul_group(ps, w2, m, h_tiles, K2)
            g = g_p.tile([P, B], F32, tag="g", name=f"g_{m}")
            if zero_bias:
                nc.scalar.activation(_mm(g[:]), ps[:], PRELU, alpha=alt[:])
            else:
                nc.scalar.activation(_mm(g[:]), ps[:], PRELU,
                                     bias=b2t[:, m:m + 1], alpha=alt[:])
            g_tiles.append(g)

        # ---- classifier: out[1, B] = sum_c Wc[c].T @ g[c] (+ bc) ----
        po = cls_ps.tile([1, B], F32)
        for c in range(KC):
            nc.tensor.matmul(
                po[:], lhsT=_mm(wc[:, c:c + 1]), rhs=_mm(g_tiles[c][:]),
                start=(c == 0), stop=(c == KC - 1),
            )
        ob = out_p.tile([1, B], F32)
        if zero_bias:
            nc.vector.tensor_copy(ob[:], po[:])
        else:
            nc.vector.tensor_scalar_add(ob[:], po[:], bct[:, 0:1])
        nc.sync.dma_start(out_d, ob[:])

    nc.compile()
    return nc


_CACHE = {}


def _get_nc(zero_bias: bool):
    if zero_bias not in _CACHE:
        _CACHE[zero_bias] = _build(zero_bias)
    return _CACHE[zero_bias]


def _run(inputs, trace=False, **kw):
    def f32(a):
        return np.ascontiguousarray(np.asarray(a), dtype=np.float32)

    x = f32(inputs["x"])
    W0, b0 = f32(inputs["W0"]), f32(inputs["b0"])
    W1, b1 = f32(inputs["W1"]), f32(inputs["b1"])
    W2, b2 = f32(inputs["W2"]), f32(inputs["b2"])
    Wc, bc = f32(inputs["Wc"]), f32(inputs["bc"])
    zero_bias = not (b0.any() or b1.any() or b2.any() or bc.any())
    nc = _get_nc(zero_bias)

    eye = np.eye(P, dtype=np.float32)
    in_maps = []
    for i in range(N_CORES):
        in_maps.append({
            "x": x[i * B:(i + 1) * B],
            "W0": W0, "b0": b0, "W1": W1, "b1": b1,
            "W2": W2, "b2": b2, "Wc": Wc, "bc": bc,
            "eye": eye,
        })
    res = run_bass_kernel_spmd(nc, in_maps, list(range(N_CORES)),
                               trace=trace, **kw)
    out = np.empty((B_FULL, 1), dtype=np.float32)
    for i in range(N_CORES):
        out[i * B:(i + 1) * B, 0] = res.results[i]["out"][0]
    return out, res


def kernel(**inputs) -> np.ndarray:
    out, _ = _run(inputs)
    return out



# revision 3
# speedup vs baseline: 1.0242x; 1.0242x over previous
"""Trainium2 Bass kernel for nn_GCN_23029614641773.

The reference GCN operates on B independent 27-node graphs where every node of
graph i starts with the same feature vector x[i], and only node 0 of each graph
feeds the classifier head. Exploiting linearity of the edge aggregation, the
whole network collapses exactly (up to fp rounding order) to a per-sample MLP:

    y = x @ W0                                  # [B, 1024]
    s = lrelu(y + b0) + 2*lrelu(3y + b0) + lrelu(5y + b0)
      # node 1's in-neighbours {0,2,4,6} have in-degrees {1,3,3,5};
      # 2*lrelu(3y+b0) == lrelu(6y+2*b0) exactly (scaling by 2 is exact).
      # With b0 == 0 (spec fill): s == max(12y, 2.4y) exactly.
    t = s @ W1;  h = lrelu(t + b1)              # [B, 512]
    v = h @ W2;  g = lrelu(v + b2)              # [B, 256]
    out = g @ Wc + bc                           # [B, 1]

Sharding: pure data parallelism, batch split across 8 NeuronCores; each core
holds the full weight set.

Perf design (v2): the kernel is HBM-DMA bound (weights dominate), so all
operands are cast to fp16 on the host (halves traffic; measured end-to-end
rel err ~7e-4, fp8 variants all exceed 2e-2). The host also pre-packs every
tensor into the exact SBUF tile layout — x arrives pre-transposed and weights
arrive as [128, m-block x k x 128] tiles — so every DMA is a flat contiguous
[128, N] copy (no descriptor-heavy rearranges, no on-chip PE transposes).
Weight m-blocks stream one DMA each, pipelined against the PE's m-block
matmul loop; fp16 also enables the PE's fast-weight-load path (fp32r does
not), roughly halving per-matmul cost.
"""

from contextlib import ExitStack

import numpy as np

import concourse.bacc as bacc
import concourse.mybir as mybir
import concourse.tile as tile
from concourse.bass_utils import run_bass_kernel_spmd

F32 = mybir.dt.float32
F16 = mybir.dt.float16
P = 128
N_CORES = 8
B_FULL = 2048
B = B_FULL // N_CORES  # 256 rows per core
D0, D1, D2, D3 = 1024, 1024, 512, 256
K0, M0 = D0 // P, D1 // P  # 8, 8
K1, M1 = D1 // P, D2 // P  # 8, 4
K2, M2 = D2 // P, D3 // P  # 4, 2
KC = D3 // P  # 2

NEG_SLOPE = 0.2


def _build(zero_bias: bool):
    nc = bacc.Bacc(
        "TRN2", target_bir_lowering=False, debug=False,
        enable_asserts=False, num_devices=1,
    )

    xt_d = nc.dram_tensor("xt", [P, K0 * B], F16, kind="ExternalInput").ap()
    w0_d = nc.dram_tensor("w0p", [P, M0 * K0 * P], F16, kind="ExternalInput").ap()
    w1_d = nc.dram_tensor("w1p", [P, M1 * K1 * P], F16, kind="ExternalInput").ap()
    w2_d = nc.dram_tensor("w2p", [P, M2 * K2 * P], F16, kind="ExternalInput").ap()
    wc_d = nc.dram_tensor("wcp", [P, KC], F16, kind="ExternalInput").ap()
    if not zero_bias:
        b0_d = nc.dram_tensor("b0", [D1], F32, kind="ExternalInput").ap()
        b1_d = nc.dram_tensor("b1", [D2], F32, kind="ExternalInput").ap()
        b2_d = nc.dram_tensor("b2", [D3], F32, kind="ExternalInput").ap()
        bc_d = nc.dram_tensor("bc", [1], F32, kind="ExternalInput").ap()
    out_d = nc.dram_tensor("out", [1, B], F32, kind="ExternalOutput").ap()

    with ExitStack() as ctx:
        tc = ctx.enter_context(tile.TileContext(nc))
        const = ctx.enter_context(tc.tile_pool(name="const", bufs=1))
        xt_p = ctx.enter_context(tc.tile_pool(name="xt", bufs=K0))
        w0_p = ctx.enter_context(tc.tile_pool(name="w0", bufs=M0))
        w1_p = ctx.enter_context(tc.tile_pool(name="w1", bufs=M1))
        w2_p = ctx.enter_context(tc.tile_pool(name="w2", bufs=M2))
        wc_p = ctx.enter_context(tc.tile_pool(name="wc", bufs=1))
        s_p = ctx.enter_context(tc.tile_pool(name="s", bufs=K1))
        h_p = ctx.enter_context(tc.tile_pool(name="h", bufs=K2))
        g_p = ctx.enter_context(tc.tile_pool(name="g", bufs=KC))
        tmp_p = ctx.enter_context(tc.tile_pool(name="tmp", bufs=4))
        out_p = ctx.enter_context(tc.tile_pool(name="outp", bufs=1))
        ps_p = ctx.enter_context(tc.tile_pool(name="ps", bufs=6, space="PSUM"))
        cls_ps = ctx.enter_context(tc.tile_pool(name="cls", bufs=1, space="PSUM"))

        # leaky-relu slope as a per-partition alpha vector for ACT Prelu
        alt = const.tile([P, 1], F32, tag="alt")
        nc.vector.memset(alt[:], NEG_SLOPE)

        # ---- DMA issue order == HBM arrival order (single HWDGE ring):
        # W0 block 0 + x first (gates the first matmul group), then the
        # remaining W0 blocks pipelined against the L1 m-loop, then W1/W2.
        # Small loads (Wc, biases) go on the scalar ring. ----
        w0 = []
        t = w0_p.tile([P, K0 * P], F16, tag="w", name="w0_0")
        nc.sync.dma_start(t[:], w0_d[:, 0:K0 * P])
        w0.append(t)
        xt = []
        for k in range(K0):
            t = xt_p.tile([P, B], F16, tag="xt", name=f"xt_{k}")
            nc.sync.dma_start(t[:], xt_d[:, k * B:(k + 1) * B])
            xt.append(t)
        for m in range(1, M0):
            t = w0_p.tile([P, K0 * P], F16, tag="w", name=f"w0_{m}")
            nc.sync.dma_start(t[:], w0_d[:, m * K0 * P:(m + 1) * K0 * P])
            w0.append(t)
        w1 = []
        for m in range(M1):
            t = w1_p.tile([P, K1 * P], F16, tag="w", name=f"w1_{m}")
            nc.sync.dma_start(t[:], w1_d[:, m * K1 * P:(m + 1) * K1 * P])
            w1.append(t)
        w2 = []
        for m in range(M2):
            t = w2_p.tile([P, K2 * P], F16, tag="w", name=f"w2_{m}")
            nc.sync.dma_start(t[:], w2_d[:, m * K2 * P:(m + 1) * K2 * P])
            w2.append(t)
        wc = wc_p.tile([P, KC], F16)
        nc.scalar.dma_start(wc[:], wc_d)

        if not zero_bias:
            b0t = const.tile([P, M0], F32, tag="b0t")
            nc.scalar.dma_start(b0t[:], b0_d.rearrange("(c p) -> p c", p=P))
            b1t = const.tile([P, M1], F32, tag="b1t")
            nc.scalar.dma_start(b1t[:], b1_d.rearrange("(c p) -> p c", p=P))
            b2t = const.tile([P, M2], F32, tag="b2t")
            nc.scalar.dma_start(b2t[:], b2_d.rearrange("(c p) -> p c", p=P))
            bct = const.tile([1, 1], F32, tag="bct")
            nc.scalar.dma_start(bct[:], bc_d.rearrange("(a b) -> a b", a=1))
            b0t2 = const.tile([P, M0], F32, tag="b0t2")
            nc.vector.tensor_scalar_mul(b0t2[:], b0t[:], 2.0)

        PRELU = mybir.ActivationFunctionType.Prelu

        def matmul_group(ps, wt, m, rhs_tiles, K):
            for k in range(K):
                nc.tensor.matmul(
                    ps[:], lhsT=wt[:, k * P:(k + 1) * P],
                    rhs=rhs_tiles[k][:],
                    start=(k == 0), stop=(k == K - 1),
                )

        # ---- layer 1: y[m] = sum_k W0[k,m].T @ xT[k];
        #      s = 12*lrelu(y) = Prelu(12*y) exactly (zero bias) ----
        s_tiles = []
        for m in range(M0):
            ps = ps_p.tile([P, B], F32, tag="ps", name=f"ps1_{m}")
            matmul_group(ps, w0[m][:], m, xt, K0)
            s = s_p.tile([P, B], F16, tag="s", name=f"s_{m}")
            if zero_bias:
                nc.scalar.activation(s[:], ps[:], PRELU, scale=12.0, alpha=alt[:])
            else:
                acc = tmp_p.tile([P, B], F32, tag="l", name=f"acc_{m}")
                first = True
                for scale, bias in ((1.0, b0t[:, m:m + 1]), (6.0, b0t2[:, m:m + 1]),
                                    (5.0, b0t[:, m:m + 1])):
                    l = tmp_p.tile([P, B], F32, tag="l", name=f"l_{m}")
                    nc.scalar.activation(l[:], ps[:], PRELU,
                                         scale=scale, bias=bias, alpha=alt[:])
                    if first:
                        nc.vector.tensor_copy(acc[:], l[:])
                        first = False
                    else:
                        nc.vector.tensor_add(acc[:], acc[:], l[:])
                nc.vector.tensor_copy(s[:], acc[:])
            s_tiles.append(s)

        # ---- layer 2: t[m] = sum_k W1[k,m].T @ s[k]; h = lrelu(t + b1) ----
        h_tiles = []
        for m in range(M1):
            ps = ps_p.tile([P, B], F32, tag="ps", name=f"ps2_{m}")
            matmul_group(ps, w1[m][:], m, s_tiles, K1)
            h = h_p.tile([P, B], F16, tag="h", name=f"h_{m}")
            if zero_bias:
                nc.scalar.activation(h[:], ps[:], PRELU, alpha=alt[:])
            else:
                nc.scalar.activation(h[:], ps[:], PRELU,
                                     bias=b1t[:, m:m + 1], alpha=alt[:])
            h_tiles.append(h)

        # ---- layer 3: v[m] = sum_k W2[k,m].T @ h[k]; g = lrelu(v + b2) ----
        g_tiles = []
        for m in range(M2):
            ps = ps_p.tile([P, B], F32, tag="ps", name=f"ps3_{m}")
            matmul_group(ps, w2[m][:], m, h_tiles, K2)
            g = g_p.tile([P, B], F16, tag="g", name=f"g_{m}")
            if zero_bias:
                nc.scalar.activation(g[:], ps[:], PRELU, alpha=alt[:])
            else:
                nc.scalar.activation(g[:], ps[:], PRELU,
                                     bias=b2t[:, m:m + 1], alpha=alt[:])
            g_tiles.append(g)

        # ---- classifier: out[1, B] = sum_c Wc[c].T @ g[c] (+ bc) ----
        po = cls_ps.tile([1, B], F32)
        for c in range(KC):
            nc.tensor.matmul(
                po[:], lhsT=wc[:, c:c + 1], rhs=g_tiles[c][:],
                start=(c == 0), stop=(c == KC - 1),
            )
        ob = out_p.tile([1, B], F32)
        if zero_bias:
            nc.vector.tensor_copy(ob[:], po[:])
        else:
            nc.vector.tensor_scalar_add(ob[:], po[:], bct[:, 0:1])
        nc.sync.dma_start(out_d, ob[:])

    nc.compile()
    return nc


_CACHE = {}


def _get_nc(zero_bias: bool):
    if zero_bias not in _CACHE:
        _CACHE[zero_bias] = _build(zero_bias)
    return _CACHE[zero_bias]


def _run(inputs, trace=False, **kw):
    def f32(a):
        return np.ascontiguousarray(np.asarray(a), dtype=np.float32)

    x = f32(inputs["x"])
    W0, b0 = f32(inputs["W0"]), f32(inputs["b0"])
    W1, b1 = f32(inputs["W1"]), f32(inputs["b1"])
    W2, b2 = f32(inputs["W2"]), f32(inputs["b2"])
    Wc, bc = f32(inputs["Wc"]), f32(inputs["bc"])
    zero_bias = not (b0.any() or b1.any() or b2.any() or bc.any())
    nc = _get_nc(zero_bias)

    # Host-side packing into exact SBUF tile layouts, fp16.
    def pack_w(W, K, M):
        return np.ascontiguousarray(
            W.astype(np.float16).reshape(K, P, M, P)
            .transpose(1, 2, 0, 3).reshape(P, M * K * P))

    w0p = pack_w(W0, K0, M0)
    w1p = pack_w(W1, K1, M1)
    w2p = pack_w(W2, K2, M2)
    wcp = np.ascontiguousarray(Wc.astype(np.float16)[:, 0].reshape(KC, P).T)

    in_maps = []
    for i in range(N_CORES):
        xs = x[i * B:(i + 1) * B].astype(np.float16)  # [256, 1024]
        xtp = np.ascontiguousarray(
            xs.T.reshape(K0, P, B).transpose(1, 0, 2).reshape(P, K0 * B))
        m = {"xt": xtp, "w0p": w0p, "w1p": w1p, "w2p": w2p, "wcp": wcp}
        if not zero_bias:
            m.update({"b0": b0, "b1": b1, "b2": b2, "bc": bc})
        in_maps.append(m)
    res = run_bass_kernel_spmd(nc, in_maps, list(range(N_CORES)),
                               trace=trace, **kw)
    out = np.empty((B_FULL, 1), dtype=np.float32)
    for i in range(N_CORES):
        out[i * B:(i + 1) * B, 0] = res.results[i]["out"][0]
    return out, res


def kernel(**inputs) -> np.ndarray:
    out, _ = _run(inputs)
    return out


# revision 14
# speedup vs baseline: 1.0770x; 1.0515x over previous
"""Trainium2 Bass kernel for nn_GCN_23029614641773.

The reference GCN operates on B independent 27-node graphs where every node of
graph i starts with the same feature vector x[i], and only node 0 of each graph
feeds the classifier head. Exploiting linearity of the edge aggregation, the
whole network collapses exactly (up to fp rounding order) to a per-sample MLP:

    y = x @ W0                                  # [B, 1024]
    s = lrelu(y + b0) + 2*lrelu(3y + b0) + lrelu(5y + b0)
      # node 1's in-neighbours {0,2,4,6} have in-degrees {1,3,3,5};
      # 2*lrelu(3y+b0) == lrelu(6y+2*b0) exactly (scaling by 2 is exact).
      # With b0 == 0 (spec fill): s == max(12y, 2.4y) exactly.
    t = s @ W1;  h = lrelu(t + b1)              # [B, 512]
    v = h @ W2;  g = lrelu(v + b2)              # [B, 256]
    out = g @ Wc + bc                           # [B, 1]

Sharding: pure data parallelism, batch split across 8 NeuronCores; each core
holds the full weight set.

Perf design (v2): the kernel is HBM-DMA bound (weights dominate), so all
operands are cast to fp16 on the host (halves traffic; measured end-to-end
rel err ~7e-4, fp8 variants all exceed 2e-2). The host also pre-packs every
tensor into the exact SBUF tile layout — x arrives pre-transposed and weights
arrive as [128, m-block x k x 128] tiles — so every DMA is a flat contiguous
[128, N] copy (no descriptor-heavy rearranges, no on-chip PE transposes).
Weight m-blocks stream one DMA each, pipelined against the PE's m-block
matmul loop; fp16 also enables the PE's fast-weight-load path (fp32r does
not), roughly halving per-matmul cost.
"""

from contextlib import ExitStack

import numpy as np

import concourse.bacc as bacc
import concourse.mybir as mybir
import concourse.tile as tile
from concourse.bass_utils import run_bass_kernel_spmd

F32 = mybir.dt.float32
F16 = mybir.dt.float16
P = 128
N_CORES = 8
B_FULL = 2048
B = B_FULL // N_CORES  # 256 rows per core
D0, D1, D2, D3 = 1024, 1024, 512, 256
K0, M0 = D0 // P, D1 // P  # 8, 8
K1, M1 = D1 // P, D2 // P  # 8, 4
K2, M2 = D2 // P, D3 // P  # 4, 2
KC = D3 // P  # 2

NEG_SLOPE = 0.2


def _build(zero_bias: bool):
    nc = bacc.Bacc(
        "TRN2", target_bir_lowering=False, debug=False,
        enable_asserts=False, num_devices=1,
    )

    xt_d = nc.dram_tensor("xt", [P, K0 * B], F16, kind="ExternalInput").ap()
    w0_d = nc.dram_tensor("w0p", [P, M0 * K0 * P], F16, kind="ExternalInput").ap()
    w1_d = nc.dram_tensor("w1p", [P, M1 * K1 * P], F16, kind="ExternalInput").ap()
    # W2 blocks and Wc ride in one packed tensor (one DMA, one tile).
    w2_d = nc.dram_tensor("w2p", [P, M2 * K2 * P + KC], F16,
                          kind="ExternalInput").ap()
    if not zero_bias:
        b0_d = nc.dram_tensor("b0", [D1], F32, kind="ExternalInput").ap()
        b1_d = nc.dram_tensor("b1", [D2], F32, kind="ExternalInput").ap()
        b2_d = nc.dram_tensor("b2", [D3], F32, kind="ExternalInput").ap()
        bc_d = nc.dram_tensor("bc", [1], F32, kind="ExternalInput").ap()
    out_d = nc.dram_tensor("out", [1, B], F32, kind="ExternalOutput").ap()

    with ExitStack() as ctx:
        tc = ctx.enter_context(tile.TileContext(nc))
        const = ctx.enter_context(tc.tile_pool(name="const", bufs=1))
        xt_p = ctx.enter_context(tc.tile_pool(name="xt", bufs=1))
        w0_p = ctx.enter_context(tc.tile_pool(name="w0", bufs=3))
        w1_p = ctx.enter_context(tc.tile_pool(name="w1", bufs=1))
        w2_p = ctx.enter_context(tc.tile_pool(name="w2", bufs=1))
        s_p = ctx.enter_context(tc.tile_pool(name="s", bufs=K1))
        h_p = ctx.enter_context(tc.tile_pool(name="h", bufs=K2))
        g_p = ctx.enter_context(tc.tile_pool(name="g", bufs=KC))
        tmp_p = ctx.enter_context(tc.tile_pool(name="tmp", bufs=4))
        out_p = ctx.enter_context(tc.tile_pool(name="outp", bufs=1))
        ps_p = ctx.enter_context(tc.tile_pool(name="ps", bufs=6, space="PSUM"))
        cls_ps = ctx.enter_context(tc.tile_pool(name="cls", bufs=1, space="PSUM"))

        # leaky-relu slope as a per-partition alpha vector for ACT Prelu
        alt = const.tile([P, 1], F32, tag="alt")
        nc.vector.memset(alt[:], NEG_SLOPE)

        # ---- DMA plan. Two HW facts drive this (measured from traces):
        # (1) HWDGE descriptor generation costs ~650ns per dma_start
        #     (128 per-partition descriptors @ ~5ns), serialized on the
        #     issuing sequencer, INDEPENDENT of transfer size.
        # (2) The 16 SDMA engines round-robin between ACTIVE QUEUES at
        #     packet granularity — concurrent queues halve per-engine
        #     throughput and let late-needed data cut in line.
        # So: ONE queue (sync HWDGE) for all bulk data, few large DMAs,
        # strictly ordered by when compute needs them. W0 is split so
        # m-block 0 lands early (PE start) while the rest stream behind
        # the L1 m-loop. ----
        xt_t = xt_p.tile([P, K0 * B], F16, tag="xt", name="xt")
        nc.sync.dma_start(xt_t[:], xt_d)
        xt = [xt_t[:, k * B:(k + 1) * B] for k in range(K0)]

        w0a = w0_p.tile([P, K0 * P], F16, tag="w", name="w0a")
        nc.sync.dma_start(w0a[:], w0_d[:, 0:K0 * P])
        w0b = w0_p.tile([P, 3 * K0 * P], F16, tag="w", name="w0b")
        nc.sync.dma_start(w0b[:], w0_d[:, K0 * P:4 * K0 * P])
        w0c = w0_p.tile([P, 4 * K0 * P], F16, tag="w", name="w0c")
        nc.sync.dma_start(w0c[:], w0_d[:, 4 * K0 * P:8 * K0 * P])
        w1t = w1_p.tile([P, M1 * K1 * P], F16, tag="w", name="w1")
        nc.sync.dma_start(w1t[:], w1_d)
        w2t = w2_p.tile([P, M2 * K2 * P + KC], F16, tag="w", name="w2")
        nc.sync.dma_start(w2t[:], w2_d)
        wc = w2t[:, M2 * K2 * P:M2 * K2 * P + KC]

        def w0_lhsT(m, k):
            if m == 0:
                return w0a[:, k * P:(k + 1) * P]
            if m < 4:
                return w0b[:, ((m - 1) * K0 + k) * P:((m - 1) * K0 + k + 1) * P]
            return w0c[:, ((m - 4) * K0 + k) * P:((m - 4) * K0 + k + 1) * P]

        def w1_lhsT(m, k):
            return w1t[:, (m * K1 + k) * P:(m * K1 + k + 1) * P]

        def w2_lhsT(m, k):
            return w2t[:, (m * K2 + k) * P:(m * K2 + k + 1) * P]

        if not zero_bias:
            b0t = const.tile([P, M0], F32, tag="b0t")
            nc.scalar.dma_start(b0t[:], b0_d.rearrange("(c p) -> p c", p=P))
            b1t = const.tile([P, M1], F32, tag="b1t")
            nc.scalar.dma_start(b1t[:], b1_d.rearrange("(c p) -> p c", p=P))
            b2t = const.tile([P, M2], F32, tag="b2t")
            nc.scalar.dma_start(b2t[:], b2_d.rearrange("(c p) -> p c", p=P))
            bct = const.tile([1, 1], F32, tag="bct")
            nc.scalar.dma_start(bct[:], bc_d.rearrange("(a b) -> a b", a=1))
            b0t2 = const.tile([P, M0], F32, tag="b0t2")
            nc.vector.tensor_scalar_mul(b0t2[:], b0t[:], 2.0)

        PRELU = mybir.ActivationFunctionType.Prelu

        def matmul_group(ps, lhsT_fn, m, rhs_tiles, K):
            for k in range(K):
                nc.tensor.matmul(
                    ps[:], lhsT=lhsT_fn(m, k),
                    rhs=rhs_tiles[k],
                    start=(k == 0), stop=(k == K - 1),
                )

        # ---- layer 1: y[m] = sum_k W0[k,m].T @ xT[k];
        #      s = 12*lrelu(y) = Prelu(12*y) exactly (zero bias) ----
        s_tiles = []
        for m in range(M0):
            ps = ps_p.tile([P, B], F32, tag="ps", name=f"ps1_{m}")
            matmul_group(ps, w0_lhsT, m, xt, K0)
            s = s_p.tile([P, B], F16, tag="s", name=f"s_{m}")
            if zero_bias:
                nc.scalar.activation(s[:], ps[:], PRELU, scale=12.0, alpha=alt[:])
            else:
                acc = tmp_p.tile([P, B], F32, tag="l", name=f"acc_{m}")
                first = True
                for scale, bias in ((1.0, b0t[:, m:m + 1]), (6.0, b0t2[:, m:m + 1]),
                                    (5.0, b0t[:, m:m + 1])):
                    l = tmp_p.tile([P, B], F32, tag="l", name=f"l_{m}")
                    nc.scalar.activation(l[:], ps[:], PRELU,
                                         scale=scale, bias=bias, alpha=alt[:])
                    if first:
                        nc.vector.tensor_copy(acc[:], l[:])
                        first = False
                    else:
                        nc.vector.tensor_add(acc[:], acc[:], l[:])
                nc.vector.tensor_copy(s[:], acc[:])
            s_tiles.append(s)

        # ---- layer 2: t[m] = sum_k W1[k,m].T @ s[k]; h = lrelu(t + b1) ----
        h_tiles = []
        for m in range(M1):
            ps = ps_p.tile([P, B], F32, tag="ps", name=f"ps2_{m}")
            matmul_group(ps, w1_lhsT, m, [t[:] for t in s_tiles], K1)
            h = h_p.tile([P, B], F16, tag="h", name=f"h_{m}")
            if zero_bias:
                nc.scalar.activation(h[:], ps[:], PRELU, alpha=alt[:])
            else:
                nc.scalar.activation(h[:], ps[:], PRELU,
                                     bias=b1t[:, m:m + 1], alpha=alt[:])
            h_tiles.append(h)

        # ---- layer 3: v[m] = sum_k W2[k,m].T @ h[k]; g = lrelu(v + b2) ----
        g_tiles = []
        for m in range(M2):
            ps = ps_p.tile([P, B], F32, tag="ps", name=f"ps3_{m}")
            matmul_group(ps, w2_lhsT, m, [t[:] for t in h_tiles], K2)
            g = g_p.tile([P, B], F16, tag="g", name=f"g_{m}")
            if zero_bias:
                nc.scalar.activation(g[:], ps[:], PRELU, alpha=alt[:])
            else:
                nc.scalar.activation(g[:], ps[:], PRELU,
                                     bias=b2t[:, m:m + 1], alpha=alt[:])
            g_tiles.append(g)

        # ---- classifier: out[1, B] = sum_c Wc[c].T @ g[c] (+ bc) ----
        po = cls_ps.tile([1, B], F32)
        for c in range(KC):
            nc.tensor.matmul(
                po[:], lhsT=wc[:, c:c + 1], rhs=g_tiles[c][:],
                start=(c == 0), stop=(c == KC - 1),
            )
        ob = out_p.tile([1, B], F32)
        if zero_bias:
            nc.vector.tensor_copy(ob[:], po[:])
        else:
            nc.vector.tensor_scalar_add(ob[:], po[:], bct[:, 0:1])
        nc.sync.dma_start(out_d, ob[:])

    nc.compile()
    return nc


_CACHE = {}


def _get_nc(zero_bias: bool):
    if zero_bias not in _CACHE:
        _CACHE[zero_bias] = _build(zero_bias)
    return _CACHE[zero_bias]


def _run(inputs, trace=False, **kw):
    def f32(a):
        return np.ascontiguousarray(np.asarray(a), dtype=np.float32)

    x = f32(inputs["x"])
    W0, b0 = f32(inputs["W0"]), f32(inputs["b0"])
    W1, b1 = f32(inputs["W1"]), f32(inputs["b1"])
    W2, b2 = f32(inputs["W2"]), f32(inputs["b2"])
    Wc, bc = f32(inputs["Wc"]), f32(inputs["bc"])
    zero_bias = not (b0.any() or b1.any() or b2.any() or bc.any())
    nc = _get_nc(zero_bias)

    # Host-side packing into exact SBUF tile layouts, fp16.
    def pack_w(W, K, M):
        return np.ascontiguousarray(
            W.astype(np.float16).reshape(K, P, M, P)
            .transpose(1, 2, 0, 3).reshape(P, M * K * P))

    w0p = pack_w(W0, K0, M0)
    w1p = pack_w(W1, K1, M1)
    wcp = Wc.astype(np.float16)[:, 0].reshape(KC, P).T  # [128, 2]
    w2p = np.ascontiguousarray(
        np.concatenate([pack_w(W2, K2, M2), wcp], axis=1))

    in_maps = []
    for i in range(N_CORES):
        xs = x[i * B:(i + 1) * B].astype(np.float16)  # [256, 1024]
        xtp = np.ascontiguousarray(
            xs.T.reshape(K0, P, B).transpose(1, 0, 2).reshape(P, K0 * B))
        m = {"xt": xtp, "w0p": w0p, "w1p": w1p, "w2p": w2p}
        if not zero_bias:
            m.update({"b0": b0, "b1": b1, "b2": b2, "bc": bc})
        in_maps.append(m)
    res = run_bass_kernel_spmd(nc, in_maps, list(range(N_CORES)),
                               trace=trace, **kw)
    out = np.empty((B_FULL, 1), dtype=np.float32)
    for i in range(N_CORES):
        out[i * B:(i + 1) * B, 0] = res.results[i]["out"][0]
    return out, res


def kernel(**inputs) -> np.ndarray:
    out, _ = _run(inputs)
    return out


# revision 19
# speedup vs baseline: 1.2336x; 1.1454x over previous
"""Trainium2 Bass kernel for nn_GCN_23029614641773.

The reference GCN operates on B independent 27-node graphs where every node of
graph i starts with the same feature vector x[i], and only node 0 of each graph
feeds the classifier head. Exploiting linearity of the edge aggregation, the
whole network collapses exactly (up to fp rounding order) to a per-sample MLP:

    y = x @ W0                                  # [B, 1024]
    s = lrelu(y + b0) + 2*lrelu(3y + b0) + lrelu(5y + b0)
      # node 1's in-neighbours {0,2,4,6} have in-degrees {1,3,3,5};
      # 2*lrelu(3y+b0) == lrelu(6y+2*b0) exactly (scaling by 2 is exact).
      # With b0 == 0 (spec fill): s == max(12y, 2.4y) exactly.
    t = s @ W1;  h = lrelu(t + b1)              # [B, 512]
    v = h @ W2;  g = lrelu(v + b2)              # [B, 256]
    out = g @ Wc + bc                           # [B, 1]

Sharding: pure data parallelism, batch split across 8 NeuronCores; each core
holds the full weight set.

Perf design (v2): the kernel is HBM-DMA bound (weights dominate), so all
operands are cast to fp16 on the host (halves traffic; measured end-to-end
rel err ~7e-4, fp8 variants all exceed 2e-2). The host also pre-packs every
tensor into the exact SBUF tile layout — x arrives pre-transposed and weights
arrive as [128, m-block x k x 128] tiles — so every DMA is a flat contiguous
[128, N] copy (no descriptor-heavy rearranges, no on-chip PE transposes).
Weight m-blocks stream one DMA each, pipelined against the PE's m-block
matmul loop; fp16 also enables the PE's fast-weight-load path (fp32r does
not), roughly halving per-matmul cost.
"""

from contextlib import ExitStack

import numpy as np

import concourse.bacc as bacc
import concourse.mybir as mybir
import concourse.tile as tile
from concourse.bass_utils import run_bass_kernel_spmd

F32 = mybir.dt.float32
F16 = mybir.dt.float16
P = 128
N_CORES = 8
B_FULL = 2048
B = B_FULL // N_CORES  # 256 rows per core
D0, D1, D2, D3 = 1024, 1024, 512, 256
K0, M0 = D0 // P, D1 // P  # 8, 8
K1, M1 = D1 // P, D2 // P  # 8, 4
K2, M2 = D2 // P, D3 // P  # 4, 2
KC = D3 // P  # 2

NEG_SLOPE = 0.2


def _build(zero_bias: bool):
    nc = bacc.Bacc(
        "TRN2", target_bir_lowering=False, debug=False,
        enable_asserts=False, num_devices=1,
    )

    # x^T and W0's m-block 0 ride in one tensor: one DMA (one completion
    # receipt) gates the first matmul group.
    xw_d = nc.dram_tensor("xw", [P, K0 * B + K0 * P], F16,
                          kind="ExternalInput").ap()
    w0_d = nc.dram_tensor("w0r", [P, (M0 - 1) * K0 * P], F16,
                          kind="ExternalInput").ap()
    w1_d = nc.dram_tensor("w1p", [P, M1 * K1 * P], F16, kind="ExternalInput").ap()
    # W2 blocks and Wc ride in one packed tensor (one DMA, one tile).
    w2_d = nc.dram_tensor("w2p", [P, M2 * K2 * P + KC], F16,
                          kind="ExternalInput").ap()
    if not zero_bias:
        b0_d = nc.dram_tensor("b0", [D1], F32, kind="ExternalInput").ap()
        b1_d = nc.dram_tensor("b1", [D2], F32, kind="ExternalInput").ap()
        b2_d = nc.dram_tensor("b2", [D3], F32, kind="ExternalInput").ap()
        bc_d = nc.dram_tensor("bc", [1], F32, kind="ExternalInput").ap()
    out_d = nc.dram_tensor("out", [1, B], F32, kind="ExternalOutput").ap()

    with ExitStack() as ctx:
        tc = ctx.enter_context(tile.TileContext(nc))
        const = ctx.enter_context(tc.tile_pool(name="const", bufs=1))
        xt_p = ctx.enter_context(tc.tile_pool(name="xt", bufs=1))
        w0_p = ctx.enter_context(tc.tile_pool(name="w0", bufs=5))
        w1_p = ctx.enter_context(tc.tile_pool(name="w1", bufs=1))
        w2_p = ctx.enter_context(tc.tile_pool(name="w2", bufs=1))
        s_p = ctx.enter_context(tc.tile_pool(name="s", bufs=K1))
        h_p = ctx.enter_context(tc.tile_pool(name="h", bufs=K2))
        g_p = ctx.enter_context(tc.tile_pool(name="g", bufs=KC))
        tmp_p = ctx.enter_context(tc.tile_pool(name="tmp", bufs=4))
        out_p = ctx.enter_context(tc.tile_pool(name="outp", bufs=1))
        ps_p = ctx.enter_context(tc.tile_pool(name="ps", bufs=6, space="PSUM"))
        cls_ps = ctx.enter_context(tc.tile_pool(name="cls", bufs=1, space="PSUM"))
        warm_ps = ctx.enter_context(tc.tile_pool(name="warm", bufs=1,
                                                 space="PSUM"))

        # leaky-relu slope as a per-partition alpha vector for ACT Prelu
        alt = const.tile([P, 1], F32, tag="alt")
        nc.vector.memset(alt[:], NEG_SLOPE)

        # ---- PE warmup: the tensor engine's clock ramps with sustained use
        # (~2x slower cold). Fill the otherwise-idle input-DMA window with a
        # zero matmul accumulation group so the array is at full p-state when
        # real data lands. One group -> no inter-matmul semaphores. ----
        NWARM = 20
        wz = const.tile([P, B], F16, tag="wz")
        nc.vector.memset(wz[:], 0.0)
        pw = warm_ps.tile([P, B], F32)
        for i in range(NWARM):
            nc.tensor.matmul(pw[:], lhsT=wz[:, 0:P], rhs=wz[:],
                             start=(i == 0), stop=(i == NWARM - 1))

        # ---- DMA plan. Two HW facts drive this (measured from traces):
        # (1) HWDGE descriptor generation costs ~650ns per dma_start
        #     (128 per-partition descriptors @ ~5ns), serialized on the
        #     issuing sequencer, INDEPENDENT of transfer size.
        # (2) The 16 SDMA engines round-robin between ACTIVE QUEUES at
        #     packet granularity — concurrent queues halve per-engine
        #     throughput and let late-needed data cut in line.
        # So: ONE queue (sync HWDGE) for all bulk data, few large DMAs,
        # strictly ordered by when compute needs them. W0 is split so
        # m-block 0 lands early (PE start) while the rest stream behind
        # the L1 m-loop. ----
        xw_t = xt_p.tile([P, K0 * B + K0 * P], F16, tag="xt", name="xw")
        nc.sync.dma_start(xw_t[:], xw_d)
        xt = [xw_t[:, k * B:(k + 1) * B] for k in range(K0)]
        w0m0 = xw_t[:, K0 * B:K0 * B + K0 * P]

        # Remaining W0 m-blocks, chunked to stay just ahead of the PE's
        # ~1us-per-m-block L1 cadence given ~1.7us DMA completion latency.
        w0rest = []
        for lo, hi in ((1, 2), (2, 3), (3, 4), (4, 6), (6, 8)):
            t = w0_p.tile([P, (hi - lo) * K0 * P], F16, tag="w",
                          name=f"w0_{lo}_{hi}")
            nc.sync.dma_start(
                t[:], w0_d[:, (lo - 1) * K0 * P:(hi - 1) * K0 * P])
            w0rest.append((lo, hi, t))
        w1t = w1_p.tile([P, M1 * K1 * P], F16, tag="w", name="w1")
        nc.sync.dma_start(w1t[:], w1_d)
        w2t = w2_p.tile([P, M2 * K2 * P + KC], F16, tag="w", name="w2")
        nc.sync.dma_start(w2t[:], w2_d)
        wc = w2t[:, M2 * K2 * P:M2 * K2 * P + KC]

        def w0_lhsT(m, k):
            if m == 0:
                return w0m0[:, k * P:(k + 1) * P]
            for lo, hi, t in w0rest:
                if lo <= m < hi:
                    off = ((m - lo) * K0 + k) * P
                    return t[:, off:off + P]
            raise AssertionError(m)

        def w1_lhsT(m, k):
            return w1t[:, (m * K1 + k) * P:(m * K1 + k + 1) * P]

        def w2_lhsT(m, k):
            return w2t[:, (m * K2 + k) * P:(m * K2 + k + 1) * P]

        if not zero_bias:
            b0t = const.tile([P, M0], F32, tag="b0t")
            nc.scalar.dma_start(b0t[:], b0_d.rearrange("(c p) -> p c", p=P))
            b1t = const.tile([P, M1], F32, tag="b1t")
            nc.scalar.dma_start(b1t[:], b1_d.rearrange("(c p) -> p c", p=P))
            b2t = const.tile([P, M2], F32, tag="b2t")
            nc.scalar.dma_start(b2t[:], b2_d.rearrange("(c p) -> p c", p=P))
            bct = const.tile([1, 1], F32, tag="bct")
            nc.scalar.dma_start(bct[:], bc_d.rearrange("(a b) -> a b", a=1))
            b0t2 = const.tile([P, M0], F32, tag="b0t2")
            nc.vector.tensor_scalar_mul(b0t2[:], b0t[:], 2.0)

        PRELU = mybir.ActivationFunctionType.Prelu

        def matmul_group(ps, lhsT_fn, m, rhs_tiles, K):
            for k in range(K):
                nc.tensor.matmul(
                    ps[:], lhsT=lhsT_fn(m, k),
                    rhs=rhs_tiles[k],
                    start=(k == 0), stop=(k == K - 1),
                )

        # ---- layer 1: y[m] = sum_k W0[k,m].T @ xT[k];
        #      s = 12*lrelu(y) = Prelu(12*y) exactly (zero bias) ----
        s_tiles = []
        for m in range(M0):
            ps = ps_p.tile([P, B], F32, tag="ps", name=f"ps1_{m}")
            matmul_group(ps, w0_lhsT, m, xt, K0)
            s = s_p.tile([P, B], F16, tag="s", name=f"s_{m}")
            if zero_bias:
                nc.scalar.activation(s[:], ps[:], PRELU, scale=12.0, alpha=alt[:])
            else:
                acc = tmp_p.tile([P, B], F32, tag="l", name=f"acc_{m}")
                first = True
                for scale, bias in ((1.0, b0t[:, m:m + 1]), (6.0, b0t2[:, m:m + 1]),
                                    (5.0, b0t[:, m:m + 1])):
                    l = tmp_p.tile([P, B], F32, tag="l", name=f"l_{m}")
                    nc.scalar.activation(l[:], ps[:], PRELU,
                                         scale=scale, bias=bias, alpha=alt[:])
                    if first:
                        nc.vector.tensor_copy(acc[:], l[:])
                        first = False
                    else:
                        nc.vector.tensor_add(acc[:], acc[:], l[:])
                nc.vector.tensor_copy(s[:], acc[:])
            s_tiles.append(s)

        # ---- layer 2: t[m] = sum_k W1[k,m].T @ s[k]; h = lrelu(t + b1) ----
        h_tiles = []
        for m in range(M1):
            ps = ps_p.tile([P, B], F32, tag="ps", name=f"ps2_{m}")
            matmul_group(ps, w1_lhsT, m, [t[:] for t in s_tiles], K1)
            h = h_p.tile([P, B], F16, tag="h", name=f"h_{m}")
            if zero_bias:
                nc.scalar.activation(h[:], ps[:], PRELU, alpha=alt[:])
            else:
                nc.scalar.activation(h[:], ps[:], PRELU,
                                     bias=b1t[:, m:m + 1], alpha=alt[:])
            h_tiles.append(h)

        # ---- layer 3: v[m] = sum_k W2[k,m].T @ h[k]; g = lrelu(v + b2) ----
        g_tiles = []
        for m in range(M2):
            ps = ps_p.tile([P, B], F32, tag="ps", name=f"ps3_{m}")
            matmul_group(ps, w2_lhsT, m, [t[:] for t in h_tiles], K2)
            g = g_p.tile([P, B], F16, tag="g", name=f"g_{m}")
            if zero_bias:
                nc.scalar.activation(g[:], ps[:], PRELU, alpha=alt[:])
            else:
                nc.scalar.activation(g[:], ps[:], PRELU,
                                     bias=b2t[:, m:m + 1], alpha=alt[:])
            g_tiles.append(g)

        # ---- classifier: out[1, B] = sum_c Wc[c].T @ g[c] (+ bc) ----
        po = cls_ps.tile([1, B], F32)
        for c in range(KC):
            nc.tensor.matmul(
                po[:], lhsT=wc[:, c:c + 1], rhs=g_tiles[c][:],
                start=(c == 0), stop=(c == KC - 1),
            )
        ob = out_p.tile([1, B], F32)
        if zero_bias:
            nc.vector.tensor_copy(ob[:], po[:])
        else:
            nc.vector.tensor_scalar_add(ob[:], po[:], bct[:, 0:1])
        nc.sync.dma_start(out_d, ob[:])

    nc.compile()
    return nc


_CACHE = {}


def _get_nc(zero_bias: bool):
    if zero_bias not in _CACHE:
        _CACHE[zero_bias] = _build(zero_bias)
    return _CACHE[zero_bias]


def _run(inputs, trace=False, **kw):
    def f32(a):
        return np.ascontiguousarray(np.asarray(a), dtype=np.float32)

    x = f32(inputs["x"])
    W0, b0 = f32(inputs["W0"]), f32(inputs["b0"])
    W1, b1 = f32(inputs["W1"]), f32(inputs["b1"])
    W2, b2 = f32(inputs["W2"]), f32(inputs["b2"])
    Wc, bc = f32(inputs["Wc"]), f32(inputs["bc"])
    zero_bias = not (b0.any() or b1.any() or b2.any() or bc.any())
    nc = _get_nc(zero_bias)

    # Host-side packing into exact SBUF tile layouts, fp16.
    def pack_w(W, K, M):
        return np.ascontiguousarray(
            W.astype(np.float16).reshape(K, P, M, P)
            .transpose(1, 2, 0, 3).reshape(P, M * K * P))

    w0p = pack_w(W0, K0, M0)
    w0r = np.ascontiguousarray(w0p[:, K0 * P:])
    w1p = pack_w(W1, K1, M1)
    wcp = Wc.astype(np.float16)[:, 0].reshape(KC, P).T  # [128, 2]
    w2p = np.ascontiguousarray(
        np.concatenate([pack_w(W2, K2, M2), wcp], axis=1))

    in_maps = []
    for i in range(N_CORES):
        xs = x[i * B:(i + 1) * B].astype(np.float16)  # [256, 1024]
        xtp = xs.T.reshape(K0, P, B).transpose(1, 0, 2).reshape(P, K0 * B)
        xwp = np.ascontiguousarray(
            np.concatenate([xtp, w0p[:, 0:K0 * P]], axis=1))
        m = {"xw": xwp, "w0r": w0r, "w1p": w1p, "w2p": w2p}
        if not zero_bias:
            m.update({"b0": b0, "b1": b1, "b2": b2, "bc": bc})
        in_maps.append(m)
    res = run_bass_kernel_spmd(nc, in_maps, list(range(N_CORES)),
                               trace=trace, **kw)
    out = np.empty((B_FULL, 1), dtype=np.float32)
    for i in range(N_CORES):
        out[i * B:(i + 1) * B, 0] = res.results[i]["out"][0]
    return out, res


def kernel(**inputs) -> np.ndarray:
    out, _ = _run(inputs)
    return out
